# revision 30
# baseline (speedup 1.0000x reference)
"""Trainium2 Bass kernel for nn_AttnBlock (GroupNorm + single-head attention over
32x32 image tokens + residual), batch 32, C=512, distributed data-parallel over
8 NeuronCores (4 images per core, no collectives).

All six GEMMs run in fp8e4 (TRN E4M3, max +-240) with perf_mode=DoubleRow:
each matmul contracts K=256 (two 128-slabs packed via 3D APs [128, 2, M]),
~2x the bf16 MAC rate.  Host pre-scales the four CxC weights by 16 so their
entries sit in fp8's normal range; the 16^2 factor is divided out in the exp
scale (scores) and the final residual STT (projection).

The kernel is software-pipelined across images: the scores matmuls of image i
are interleaved with the q/k/v projections of image i+1 so the PE keeps
streaming while the ACT engine works through the (slower) exp evictions of
image i; groupnorm of image i+2 is prefetched in the same iteration.

Per-image math (fp8 inputs / fp32 PSUM accumulate):
  x[c,n] --groupnorm--> hn[c,n] (fp8)
  q[o,n] = 16wq @ hn ; k[o,m] = 16wk @ hn        (lhsT = host-transposed weights)
  vT[m,c] = hn^T @ (16wv)^T                       (produced pre-transposed)
  sT[m,n] = k^T q ; eT = fp8(exp(sT/(256 sqrt(C)) - 3))   (shift 3 keeps eT in
                                                   fp8 range; softmax is shift
                                                   invariant so it cancels)
  rowsum[n] = ones^T @ eT     (DoubleRow ones-matmul; every output partition =
                               rowsum -> free broadcast)
  out[c,n] = fp8((vT^T @ eT) / rowsum)            (normalized before fp8 so the
                                                   proj input stays in range)
  y = x + (16wp @ out) / 256                      (bp is added on the host)
"""

import os
import sys

import numpy as np

for _p in ("/opt/trn_rl_repo", "/root/.axon_site/_ro/trn_rl_repo"):
    if os.path.isdir(_p) and _p not in sys.path:
        sys.path.append(_p)

from contextlib import ExitStack

import ml_dtypes  # noqa: E402

import concourse.tile as tile  # noqa: E402
from concourse import bacc, mybir  # noqa: E402
from concourse.bass_utils import run_bass_kernel_spmd  # noqa: E402

P = 128
B, C, H, W = 32, 512, 32, 32
N = H * W                  # 1024 tokens per image
CO = C // P                # 4 channel slabs of 128
FD = 512                   # matmul free-dim chunk (one PSUM bank of fp32)
NCH = N // FD              # 2 free-dim chunks
MO = N // P                # 8 token slabs of 128
GROUPS = 16
EPS = 1e-6
NCORES = 8
IPC = B // NCORES          # images per core
F32 = mybir.dt.float32
F16 = mybir.dt.float16
F8 = mybir.dt.float8e4
AF = mybir.ActivationFunctionType
OP = mybir.AluOpType
DR = mybir.MatmulPerfMode.DoubleRow
WS = 16.0                  # host-side weight scale into fp8
SC2 = float(C) ** -0.5 / (WS * WS)   # exp scale on raw q16.k16 scores
ESH = 3.0                  # exp shift (softmax-invariant; keeps eT < ~60)
FS = 1.0 / (WS * WS)       # final projection descale


def _emit(tc: "tile.TileContext", ctx: ExitStack, aps: dict):
    nc = tc.nc

    const = ctx.enter_context(tc.tile_pool(name="const", bufs=1))
    xs = ctx.enter_context(tc.tile_pool(name="xs", bufs=3))
    hns = ctx.enter_context(tc.tile_pool(name="hns", bufs=2))
    qs = ctx.enter_context(tc.tile_pool(name="qs", bufs=2))
    ks = ctx.enter_context(tc.tile_pool(name="ks", bufs=2))
    vs = ctx.enter_context(tc.tile_pool(name="vs", bufs=2))
    es = ctx.enter_context(tc.tile_pool(name="es", bufs=2))
    ous = ctx.enter_context(tc.tile_pool(name="ous", bufs=1))
    ris = ctx.enter_context(tc.tile_pool(name="ris", bufs=2))
    ys = ctx.enter_context(tc.tile_pool(name="ys", bufs=3))
    stat = ctx.enter_context(tc.tile_pool(name="stat", bufs=2))
    mmp = ctx.enter_context(tc.tile_pool(name="mmp", bufs=3, space="PSUM"))
    aux = ctx.enter_context(tc.tile_pool(name="aux", bufs=2, space="PSUM"))

    # ---- constants: one packed DMA on the GpSimd queue so the Sync queue
    # is free for the critical-path x slabs ----
    ones_sb = const.tile([P, P], F16, tag="ones")
    nc.vector.memset(ones_sb[:], 1.0)
    ones8_sb = const.tile([P, 2, P], F8, tag="ones8")
    nc.vector.memset(ones8_sb[:], 1.0)
    esh_sb = const.tile([P, 1], F32, tag="esh")
    nc.vector.memset(esh_sb[:], -ESH)
    cpack = const.tile([P, 4 * CO + P + 2 * C], F32, tag="cpack")
    nc.gpsimd.dma_start(cpack[:], aps["cpack"])
    small = {}
    for i, name in enumerate(("bq", "bk", "gamma", "beta")):
        small[name] = cpack[:, i * CO : (i + 1) * CO]
    bvb_sb = cpack[:, 4 * CO + P :]          # 16*bv tiled twice: [P, 2C]
    proj16_sb = const.tile([P, P], F16, tag="proj16")
    nc.vector.tensor_copy(proj16_sb[:], cpack[:, 4 * CO : 4 * CO + P])

    # Dummy matmuls while groupnorm owns the critical path: PE is idle anyway
    # and sustained activity lifts the HAM clock gate to 8/8 before real work.
    # The sink lives in an mmp slot (it is the pool's first rotation entry);
    # the aux pool is left to the groupnorm stats and the rowsum chunks.
    wps = mmp.tile([P, 2 * FD], F32, tag="mm")

    def warmup(n):
        for i in range(n):
            nc.tensor.matmul(
                wps[:, 0:P], lhsT=ones_sb[:], rhs=ones_sb[:], start=(i == 0), stop=(i == n - 1)
            )

    w_sb = {}

    def load_weights():
        # Emitted after prep(0) so x(0) slabs go first on the DMA queue;
        # wqT leads since the first projection matmuls consume it.
        for name in ("wqT", "wkT", "wvT", "wpT"):
            t = const.tile([P, CO, C], F8, tag=name)
            nc.sync.dma_start(t[:], aps[name].rearrange("(co ci) o -> ci co o", ci=P))
            w_sb[name] = t

    # Per-image state carried between the pipeline stages below.
    st = [dict() for _ in range(IPC)]

    def prep(img):
        """x DMA + groupnorm -> hn.

        sum(x) runs on the otherwise-idle GpSimd engine and sum(x^2) on ACT
        (Square + free-dim accumulator) so DVE keeps its bandwidth for the
        eviction stream.  rstd = 1/sqrt(var+eps) runs on DVE (quake-style
        rsqrt + Newton) so the ACT engine only ever needs one activation
        table (exp/copy/identity/square) -> one table load.
        """
        fast = img == 0  # image 0 minimizes serial latency (DVE reduce); later
        #                  images keep DVE free for the eviction stream
        x_ap = aps["x"][img].rearrange("(co ci) n -> ci co n", ci=P)
        x_sb = xs.tile([P, CO, N], F32, tag="x")
        stats = stat.tile([P, 2 * CO], F32, tag="stats")
        for co in range(CO):
            nc.sync.dma_start(x_sb[:, co], x_ap[:, co])
            if fast:
                nc.vector.reduce_sum(
                    stats[:, co : co + 1], x_sb[:, co], axis=mybir.AxisListType.X
                )
            else:
                scr = stat.tile([P, N], F16, tag="sqscr")
                nc.scalar.activation(
                    scr[:],
                    x_sb[:, co],
                    AF.Identity,
                    accum_out=stats[:, co : co + 1],
                )
            scr = stat.tile([P, N], F16, tag="sqscr")
            nc.scalar.activation(
                scr[:],
                x_sb[:, co],
                AF.Square,
                accum_out=stats[:, CO + co : CO + co + 1],
            )
        st[img]["x"] = x_sb
        st[img]["stats"] = stats

    def prep_finish(img):
        """Group stats -> rstd -> hn.  Emitted at iteration end so the tiny
        projector matmul sits in the PE queue long after its inputs are ready
        (mid-tail placement measured a 5us PE stall waiting on the stats)."""
        fast = img == 0
        x_sb, stats = st[img]["x"], st[img]["stats"]
        # stats cast to fp16 for the projector matmul (1 cyc/row vs fp32's
        # dual-pass quarter rate); group averaging divides the fp16 rounding
        # by sqrt(32), so the rstd error stays ~1e-5.
        stats16 = stat.tile([P, 2 * CO], F16, tag="stats16")
        (nc.vector if fast else nc.gpsimd).tensor_scalar(
            out=stats16[:], in0=stats[:], scalar1=1.0 / N, scalar2=None, op0=OP.mult
        )
        gs_ps = aux.tile([P, FD], F32, tag="aux")
        nc.tensor.matmul(
            gs_ps[:, 0 : 2 * CO], lhsT=proj16_sb[:], rhs=stats16[:], start=True, stop=True
        )
        m2 = stat.tile([P, CO], F32, tag="m2")
        nc.scalar.activation(m2[:], gs_ps[:, 0:CO], AF.Square)
        if fast:
            gs_sb = gs_ps          # DVE reads the group stats straight from PSUM
            eng = nc.vector
        else:
            # GpSimd has no PSUM port: ACT parks the (tiny) group stats in SBUF
            # and the whole rstd chain runs on GpSimd/ACT, which are idle at the
            # end of an iteration -- DVE is still draining evictions then.
            gs_sb = stat.tile([P, 2 * CO], F32, tag="gs_sb")
            nc.scalar.activation(gs_sb[:], gs_ps[:, 0 : 2 * CO], AF.Copy)
            eng = nc.gpsimd
        # ve and the integer quake-rsqrt seed run on DVE (the Pool engine has
        # no int ALU opcodes): 3 tiny ops, emitted mid-tail so the DVE queue
        # reaches them right after the out evictions, before the final STTs.
        ve = stat.tile([P, CO], F32, tag="ve")
        nc.vector.scalar_tensor_tensor(
            out=ve[:],
            in0=gs_sb[:, CO : 2 * CO],
            scalar=EPS,
            in1=m2[:],
            op0=OP.add,
            op1=OP.subtract,
        )
        y0i = stat.tile([P, CO], mybir.dt.int32, tag="y0i")
        nc.vector.tensor_scalar(
            out=y0i[:],
            in0=ve[:].bitcast(mybir.dt.int32),
            scalar1=1,
            scalar2=None,
            op0=OP.arith_shift_right,
        )
        nc.vector.tensor_scalar(
            out=y0i[:],
            in0=y0i[:],
            scalar1=-1,
            scalar2=0x5F3759DF,
            op0=OP.mult,
            op1=OP.add,
        )
        rstd = y0i[:].bitcast(F32)
        yy = stat.tile([P, CO], F32, tag="yy")
        eng.tensor_mul(yy[:], rstd, rstd)
        eng.tensor_mul(yy[:], yy[:], ve[:])
        eng.tensor_scalar(
            out=yy[:], in0=yy[:], scalar1=-0.5, scalar2=1.5, op0=OP.mult, op1=OP.add
        )
        nxt = stat.tile([P, CO], F32, tag="rstd")
        eng.tensor_mul(nxt[:], rstd, yy[:])
        rstd = nxt[:]
        a_sc = stat.tile([P, CO], F32, tag="a_sc")
        eng.tensor_mul(a_sc[:], small["gamma"][:], rstd[:])
        bt = stat.tile([P, CO], F32, tag="bt")
        eng.tensor_mul(bt[:], gs_sb[:, 0:CO], a_sc[:])
        b_sc = stat.tile([P, CO], F32, tag="b_sc")
        eng.tensor_sub(b_sc[:], small["beta"][:], bt[:])

        # normalize: steady-state images split ACT/GpSimd (both idle at the end
        # of an iteration; DVE is not); image 0 alternates DVE/ACT for latency.
        hn = hns.tile([P, CO, N], F8, tag="hn")
        for co in range(CO):
            if not fast:
                if co < 2:
                    nc.scalar.activation(
                        hn[:, co],
                        x_sb[:, co],
                        AF.Identity,
                        bias=b_sc[:, co : co + 1],
                        scale=a_sc[:, co : co + 1],
                    )
                else:
                    nc.gpsimd.tensor_scalar(
                        out=hn[:, co],
                        in0=x_sb[:, co],
                        scalar1=a_sc[:, co : co + 1],
                        scalar2=b_sc[:, co : co + 1],
                        op0=OP.mult,
                        op1=OP.add,
                    )
            elif co % 2 == 0:
                nc.vector.tensor_scalar(
                    out=hn[:, co],
                    in0=x_sb[:, co],
                    scalar1=a_sc[:, co : co + 1],
                    scalar2=b_sc[:, co : co + 1],
                    op0=OP.mult,
                    op1=OP.add,
                )
            else:
                nc.scalar.activation(
                    hn[:, co],
                    x_sb[:, co],
                    AF.Identity,
                    bias=b_sc[:, co : co + 1],
                    scale=a_sc[:, co : co + 1],
                )
        st[img]["hn"] = hn

    def qkv_blocks(img):
        """12 PE blocks (4 MMs each) producing q, k, vT for `img`; returned as
        closures so they can interleave with the previous image's scores."""
        hn = st[img]["hn"]
        q_sb = qs.tile([P, CO, N], F8, tag="q")
        k_sb = ks.tile([P, CO, N], F8, tag="k")
        vT = vs.tile([P, MO, C], F8, tag="vT")
        st[img]["q"] = q_sb
        st[img]["k"] = k_sb
        st[img]["vT"] = vT
        blocks = []

        def proj_block(wname, dst, ot, bname):
            def emit():
                wt = w_sb[wname]
                ps = mmp.tile([P, 2 * FD], F32, tag="mm")
                # j outer / ch inner: each DoubleRow weight pair serves two
                # moving chunks back-to-back.
                for j in range(CO // 2):
                    for ch in range(NCH):
                        nc.tensor.matmul(
                            ps[:, ch * FD : (ch + 1) * FD],
                            lhsT=wt[:, 2 * j : 2 * j + 2, ot * P : (ot + 1) * P],
                            rhs=hn[:, 2 * j : 2 * j + 2, ch * FD : (ch + 1) * FD],
                            start=(j == 0),
                            stop=(j == CO // 2 - 1),
                            perf_mode=DR,
                        )
                # evictions stay off ACT: during the scores window ACT must run
                # nothing but exps or the PSUM tile rotation stalls the PE
                nc.vector.tensor_scalar(
                    out=dst[:, ot],
                    in0=ps[:],
                    scalar1=small[bname][:, ot : ot + 1],
                    scalar2=None,
                    op0=OP.add,
                )
            return emit

        def v_block(mp):
            def emit():
                ps = mmp.tile([P, 2 * FD], F32, tag="mm")
                for h in range(2):
                    mt = 2 * mp + h
                    for j in range(CO // 2):
                        nc.tensor.matmul(
                            ps[:, h * FD : (h + 1) * FD],
                            lhsT=hn[:, 2 * j : 2 * j + 2, mt * P : (mt + 1) * P],
                            rhs=w_sb["wvT"][:, 2 * j : 2 * j + 2, :],
                            start=(j == 0),
                            stop=(j == CO // 2 - 1),
                            perf_mode=DR,
                        )
                nc.vector.tensor_add(vT[:, 2 * mp : 2 * mp + 2], ps[:], bvb_sb[:])
            return emit

        for ot in range(CO):
            blocks.append(proj_block("wqT", q_sb, ot, "bq"))
        for ot in range(CO):
            blocks.append(proj_block("wkT", k_sb, ot, "bk"))
        for mp in range(MO // 2):
            blocks.append(v_block(mp))
        return blocks

    def scores_exp(img, filler):
        """Score matmuls + exp evictions for `img`, with `filler` blocks (the
        next image's q/k/v projections) interleaved so the PE keeps streaming
        while ACT works through the exps."""
        q_sb, k_sb = st[img]["q"], st[img]["k"]
        eT = es.tile([P, MO, N], F8, tag="eT")
        fi = 0
        for mt in range(MO):
            ps = mmp.tile([P, 2 * FD], F32, tag="mm")
            for j in range(CO // 2):
                for ch in range(NCH):
                    nc.tensor.matmul(
                        ps[:, ch * FD : (ch + 1) * FD],
                        lhsT=k_sb[:, 2 * j : 2 * j + 2, mt * P : (mt + 1) * P],
                        rhs=q_sb[:, 2 * j : 2 * j + 2, ch * FD : (ch + 1) * FD],
                        start=(j == 0),
                        stop=(j == CO // 2 - 1),
                        perf_mode=DR,
                    )
            nc.scalar.activation(eT[:, mt], ps[:], AF.Exp, scale=SC2, bias=esh_sb[:])
            while fi * MO < len(filler) * (mt + 1):
                filler[fi]()
                fi += 1
        assert fi == len(filler)
        st[img]["eT"] = eT

    def tail_out(img):
        """rowsum + out GEMM; the reciprocal and the rinv-normalizing
        evictions overlap the later out blocks."""
        vT, eT = st[img]["vT"], st[img]["eT"]
        rinv = ris.tile([P, N], F32, tag="rinv")
        out8 = ous.tile([P, CO, N], F8, tag="out")
        # rowsum first (it only needs the exps, which the out matmuls need
        # anyway): per-partition-broadcast column sums of eT via DoubleRow
        # ones-matmuls.  rinv is then ready before the first out block ends,
        # so the eviction chain starts as early as possible.
        for ch in range(NCH):
            rs = aux.tile([P, FD], F32, tag="aux")
            for jm in range(MO // 2):
                nc.tensor.matmul(
                    rs[:],
                    lhsT=ones8_sb[:],
                    rhs=eT[:, 2 * jm : 2 * jm + 2, ch * FD : (ch + 1) * FD],
                    start=(jm == 0),
                    stop=(jm == MO // 2 - 1),
                    perf_mode=DR,
                )
            nc.vector.reciprocal_approx_fast(rinv[:, ch * FD : (ch + 1) * FD], rs[:])
        pso = {}
        for ct in range(CO):
            ps = mmp.tile([P, 2 * FD], F32, tag="mm")
            pso[ct] = ps
            for jm in range(MO // 2):
                for ch in range(NCH):
                    nc.tensor.matmul(
                        ps[:, ch * FD : (ch + 1) * FD],
                        lhsT=vT[:, 2 * jm : 2 * jm + 2, ct * P : (ct + 1) * P],
                        rhs=eT[:, 2 * jm : 2 * jm + 2, ch * FD : (ch + 1) * FD],
                        start=(jm == 0),
                        stop=(jm == MO // 2 - 1),
                        perf_mode=DR,
                    )
            if ct >= 1:
                # evict ct-1 while ct's matmuls stream (rinv is ready by now)
                nc.vector.tensor_mul(out8[:, ct - 1], pso[ct - 1][:], rinv[:])
        nc.vector.tensor_mul(out8[:, CO - 1], pso[CO - 1][:], rinv[:])
        st[img]["out8"] = out8

    def tail_proj(img):
        """proj GEMM + residual + store."""
        x_sb, out8 = st[img]["x"], st[img]["out8"]
        y_ap = aps["y"][img].rearrange("(co ci) n -> ci co n", ci=P)
        for ot in range(CO):
            # The very last block of the whole kernel is evicted in two
            # 512-wide halves so the exposed eviction+DMA chain after the
            # final matmul is half as long.
            last = img == IPC - 1 and ot == CO - 1
            ps = mmp.tile([P, 2 * FD], F32, tag="mm")
            if last:
                loops = [(ch, j) for ch in range(NCH) for j in range(CO // 2)]
            else:
                loops = [(ch, j) for j in range(CO // 2) for ch in range(NCH)]
            for ch, j in loops:
                nc.tensor.matmul(
                    ps[:, ch * FD : (ch + 1) * FD],
                    lhsT=w_sb["wpT"][:, 2 * j : 2 * j + 2, ot * P : (ot + 1) * P],
                    rhs=out8[:, 2 * j : 2 * j + 2, ch * FD : (ch + 1) * FD],
                    start=(j == 0),
                    stop=(j == CO // 2 - 1),
                    perf_mode=DR,
                )
            if last:
                for ch in range(NCH):
                    t2 = ys.tile([P, FD], F32, tag="yo2")
                    nc.vector.scalar_tensor_tensor(
                        out=t2[:],
                        in0=ps[:, ch * FD : (ch + 1) * FD],
                        scalar=FS,
                        in1=x_sb[:, ot, ch * FD : (ch + 1) * FD],
                        op0=OP.mult,
                        op1=OP.add,
                    )
                    nc.sync.dma_start(y_ap[:, ot, ch * FD : (ch + 1) * FD], t2[:])
            else:
                t2 = ys.tile([P, 2 * FD], F32, tag="yo")
                nc.vector.scalar_tensor_tensor(
                    out=t2[:],
                    in0=ps[:],
                    scalar=FS,
                    in1=x_sb[:, ot],
                    op0=OP.mult,
                    op1=OP.add,
                )
                nc.sync.dma_start(y_ap[:, ot], t2[:])

    prep(0)
    # 155 dummies ≈ 12us: ends right as the groupnorm stats for image 0 land,
    # so the PE flows from warmup into the stats matmul with <3.4us of idle
    # (no HAM re-throttle).  The warmup-sink eviction is emitted only after
    # prep_finish(0)'s ACT work: putting it earlier blocks the ACT FIFO until
    # the last warmup matmul retires (measured hn(0) 5us late from that).
    warmup(155)
    prep_finish(0)
    wsb = stat.tile([P, P], F32, tag="warm_sb")
    nc.scalar.activation(wsb[:], wps[:, 0:P], AF.Copy)  # releases the warm slot
    nc.gpsimd.dma_start(aps["wsink"], wsb[:])
    load_weights()
    prep(1)
    blocks0 = qkv_blocks(0)
    for blk in blocks0[:6]:
        blk()
    prep_finish(1)   # stats matmul mid-qkv: inputs ready, hn(1) early
    for blk in blocks0[6:]:
        blk()
    for img in range(IPC):
        filler = qkv_blocks(img + 1) if img + 1 < IPC else []
        scores_exp(img, filler)
        if img + 2 < IPC:
            prep(img + 2)
        tail_out(img)
        if img + 2 < IPC:
            # emitted between the out and proj phases: by the time the PE
            # reaches the tiny stats matmul the ACT sums are done, and the
            # DVE seed ops queue right behind the out evictions
            prep_finish(img + 2)
        tail_proj(img)


def _build_program():
    nc = bacc.Bacc("TRN2", target_bir_lowering=False, debug=False)
    aps = {}
    aps["x"] = nc.dram_tensor("x", [IPC, C, N], F32, kind="ExternalInput").ap()
    for name in ("wqT", "wkT", "wvT", "wpT"):
        aps[name] = nc.dram_tensor(name, [C, C], F8, kind="ExternalInput").ap()
    aps["cpack"] = nc.dram_tensor(
        "cpack", [P, 4 * CO + P + 2 * C], F32, kind="ExternalInput"
    ).ap()
    aps["y"] = nc.dram_tensor("y", [IPC, C, N], F32, kind="ExternalOutput").ap()
    aps["wsink"] = nc.dram_tensor("wsink", [P, P], F32, kind="ExternalOutput").ap()

    with tile.TileContext(nc) as tc:
        with ExitStack() as ctx:
            _emit(tc, ctx, aps)
    nc.compile()
    return nc


_PROGRAM = None


def _get_program():
    global _PROGRAM
    if _PROGRAM is None:
        _PROGRAM = _build_program()
    return _PROGRAM


def _col_layout(v):
    # (C,) vector -> [128, CO] tile layout with c = co*128 + ci at [ci, co]
    return np.ascontiguousarray(v.reshape(CO, P).T.astype(np.float32))


def _q8(w):
    return np.clip(w, -240.0, 240.0).astype(ml_dtypes.float8_e4m3)


def _make_in_maps(inputs):
    x = np.asarray(inputs["x"], dtype=np.float32).reshape(B, C, N)
    cpack = np.concatenate(
        [
            _col_layout(WS * np.asarray(inputs["bq"])),
            _col_layout(WS * np.asarray(inputs["bk"])),
            _col_layout(np.asarray(inputs["gn_gamma"])),
            _col_layout(np.asarray(inputs["gn_beta"])),
            _make_proj(),
            np.tile(
                WS * np.asarray(inputs["bv"], dtype=np.float32)[None, :], (P, 2)
            ),
        ],
        axis=1,
    )
    shared = {
        "wqT": np.ascontiguousarray(_q8(WS * np.asarray(inputs["wq"]).T)),
        "wkT": np.ascontiguousarray(_q8(WS * np.asarray(inputs["wk"]).T)),
        "wvT": np.ascontiguousarray(_q8(WS * np.asarray(inputs["wv"]).T)),
        "wpT": np.ascontiguousarray(_q8(WS * np.asarray(inputs["wp"]).T)),
        "cpack": np.ascontiguousarray(cpack),
    }
    in_maps = []
    for core in range(NCORES):
        m = dict(shared)
        m["x"] = np.ascontiguousarray(x[core * IPC : (core + 1) * IPC])
        in_maps.append(m)
    return in_maps


def _make_proj():
    # [128,128] group-averaging projector: P[i,j] = (i//32 == j//32) / 32
    # (channel c = co*128 + ci; each co slab holds 4 groups of 32 channels).
    # The kernel pre-scales the (sum, sumsq) stats by 1/N before this matmul,
    # and the fp16 copy of this matrix needs 1/32 to stay in normal range.
    gsz = P // (GROUPS // CO)  # 32
    idx = np.arange(P) // gsz
    return np.ascontiguousarray((idx[:, None] == idx[None, :]).astype(np.float32) / gsz)


def _run(inputs, trace=False):
    nc = _get_program()
    in_maps = _make_in_maps(inputs)
    res = run_bass_kernel_spmd(nc, in_maps, core_ids=list(range(NCORES)), trace=trace)
    y = np.concatenate([r["y"] for r in res.results], axis=0)  # (B, C, N)
    bp = np.asarray(inputs["bp"], dtype=np.float32)
    if np.any(bp):
        y = y + bp[None, :, None]
    return y.reshape(B, C, H, W).astype(np.float32), res.exec_time_ns


def kernel(**inputs):
    return _run(inputs, trace=False)[0]


# revision 38
# speedup vs baseline: 1.1836x; 1.1836x over previous
"""Trainium2 Bass kernel for nn_AttnBlock (GroupNorm + single-head attention over
32x32 image tokens + residual), batch 32, C=512, distributed data-parallel over
8 NeuronCores (4 images per core, no collectives).

All six GEMMs run in fp8e4 (TRN E4M3, max +-240) with perf_mode=DoubleRow:
each matmul contracts K=256 (two 128-slabs packed via 3D APs [128, 2, M]),
~2x the bf16 MAC rate.  Host pre-scales the four CxC weights by 16 so their
entries sit in fp8's normal range; the 16^2 factor is divided out in the exp
scale (scores) and the final residual STT (projection).

The kernel is software-pipelined across images: the scores matmuls of image i
are interleaved with the q/k/v projections of image i+1 so the PE keeps
streaming while the ACT engine works through the (slower) exp evictions of
image i; groupnorm of image i+2 is prefetched in the same iteration.

Per-image math (fp8 inputs / fp32 PSUM accumulate):
  x[c,n] --groupnorm--> hn[c,n] (fp8)
  q[o,n] = 16wq @ hn ; k[o,m] = 16wk @ hn        (lhsT = host-transposed weights)
  vT[m,c] = hn^T @ (16wv)^T                       (produced pre-transposed)
  sT[m,n] = k^T q ; eT = fp8(exp(sT/(256 sqrt(C)) - 3))   (shift 3 keeps eT in
                                                   fp8 range; softmax is shift
                                                   invariant so it cancels)
  rowsum[n] = ones^T @ eT     (DoubleRow ones-matmul; every output partition =
                               rowsum -> free broadcast)
  out[c,n] = fp8((vT^T @ eT) / rowsum)            (normalized before fp8 so the
                                                   proj input stays in range)
  y = x + (16wp @ out) / 256                      (bp is added on the host)
"""

import os
import sys

import numpy as np

for _p in ("/opt/trn_rl_repo", "/root/.axon_site/_ro/trn_rl_repo"):
    if os.path.isdir(_p) and _p not in sys.path:
        sys.path.append(_p)

from contextlib import ExitStack

import ml_dtypes  # noqa: E402

import concourse.tile as tile  # noqa: E402
from concourse import bacc, mybir  # noqa: E402
from concourse.bass_utils import run_bass_kernel_spmd  # noqa: E402

P = 128
B, C, H, W = 32, 512, 32, 32
N = H * W                  # 1024 tokens per image
CO = C // P                # 4 channel slabs of 128
FD = 512                   # matmul free-dim chunk (one PSUM bank of fp32)
NCH = N // FD              # 2 free-dim chunks
MO = N // P                # 8 token slabs of 128
GROUPS = 16
EPS = 1e-6
NCORES = 8
IPC = B // NCORES          # images per core
F32 = mybir.dt.float32
F16 = mybir.dt.float16
F8 = mybir.dt.float8e4
AF = mybir.ActivationFunctionType
OP = mybir.AluOpType
DR = mybir.MatmulPerfMode.DoubleRow
WS = 16.0                  # host-side weight scale into fp8
SC2 = float(C) ** -0.5 / (WS * WS)   # exp scale on raw q16.k16 scores
ESH = 3.0                  # exp shift (softmax-invariant; keeps eT < ~60)
FS = 1.0 / (WS * WS)       # final projection descale


def _emit(tc: "tile.TileContext", ctx: ExitStack, aps: dict):
    nc = tc.nc

    const = ctx.enter_context(tc.tile_pool(name="const", bufs=1))
    xs = ctx.enter_context(tc.tile_pool(name="xs", bufs=3))
    hns = ctx.enter_context(tc.tile_pool(name="hns", bufs=2))
    qs = ctx.enter_context(tc.tile_pool(name="qs", bufs=2))
    ks = ctx.enter_context(tc.tile_pool(name="ks", bufs=2))
    vs = ctx.enter_context(tc.tile_pool(name="vs", bufs=2))
    es = ctx.enter_context(tc.tile_pool(name="es", bufs=2))
    ous = ctx.enter_context(tc.tile_pool(name="ous", bufs=1))
    ris = ctx.enter_context(tc.tile_pool(name="ris", bufs=2))
    ys = ctx.enter_context(tc.tile_pool(name="ys", bufs=3))
    stat = ctx.enter_context(tc.tile_pool(name="stat", bufs=2))
    mmp = ctx.enter_context(tc.tile_pool(name="mmp", bufs=3, space="PSUM"))
    aux = ctx.enter_context(tc.tile_pool(name="aux", bufs=2, space="PSUM"))

    # ---- constants: one packed DMA on the GpSimd queue so the Sync queue
    # is free for the critical-path x slabs ----
    ones_sb = const.tile([P, P], F16, tag="ones")
    nc.vector.memset(ones_sb[:], 1.0)
    ones8_sb = const.tile([P, 2, P], F8, tag="ones8")
    nc.vector.memset(ones8_sb[:], 1.0)
    esh_sb = const.tile([P, 1], F32, tag="esh")
    nc.vector.memset(esh_sb[:], -ESH)
    cpack = const.tile([P, 4 * CO + P + 2 * C], F32, tag="cpack")
    nc.gpsimd.dma_start(cpack[:], aps["cpack"])
    small = {}
    for i, name in enumerate(("bq", "bk", "gamma", "beta")):
        small[name] = cpack[:, i * CO : (i + 1) * CO]
    bvb_sb = cpack[:, 4 * CO + P :]          # 16*bv tiled twice: [P, 2C]
    proj16_sb = const.tile([P, P], F16, tag="proj16")
    nc.vector.tensor_copy(proj16_sb[:], cpack[:, 4 * CO : 4 * CO + P])

    # Dummy matmuls while groupnorm owns the critical path: PE is idle anyway
    # and sustained activity lifts the HAM clock gate to 8/8 before real work.
    # The sink lives in an mmp slot (it is the pool's first rotation entry);
    # the aux pool is left to the groupnorm stats and the rowsum chunks.
    wps = mmp.tile([P, 2 * FD], F32, tag="mm")

    def warmup(n):
        for i in range(n):
            nc.tensor.matmul(
                wps[:, 0:P], lhsT=ones_sb[:], rhs=ones_sb[:], start=(i == 0), stop=(i == n - 1)
            )

    w_sb = {}

    def load_weights():
        # Emitted after prep(0) so x(0) slabs go first on the DMA queue;
        # wqT leads since the first projection matmuls consume it.
        for name in ("wqT", "wkT", "wvT", "wpT"):
            t = const.tile([P, CO, C], F8, tag=name)
            nc.sync.dma_start(t[:], aps[name].rearrange("(co ci) o -> ci co o", ci=P))
            w_sb[name] = t

    # Per-image state carried between the pipeline stages below.
    st = [dict() for _ in range(IPC)]

    def prep(img):
        """x DMA + groupnorm -> hn.

        sum(x) runs on the otherwise-idle GpSimd engine and sum(x^2) on ACT
        (Square + free-dim accumulator) so DVE keeps its bandwidth for the
        eviction stream.  rstd = 1/sqrt(var+eps) runs on DVE (quake-style
        rsqrt + Newton) so the ACT engine only ever needs one activation
        table (exp/copy/identity/square) -> one table load.
        """
        x_ap = aps["x"][img].rearrange("(co ci) n -> ci co n", ci=P)
        x_sb = xs.tile([P, CO, N], F32, tag="x")
        stats = stat.tile([P, 2 * CO], F32, tag="stats")
        for co in range(CO):
            nc.sync.dma_start(x_sb[:, co], x_ap[:, co])
            nc.vector.reduce_sum(
                stats[:, co : co + 1], x_sb[:, co], axis=mybir.AxisListType.X
            )
            scr = stat.tile([P, N], F16, tag="sqscr")
            nc.scalar.activation(
                scr[:],
                x_sb[:, co],
                AF.Square,
                accum_out=stats[:, CO + co : CO + co + 1],
            )
        st[img]["x"] = x_sb
        st[img]["stats"] = stats

    def prep_finish(img):
        """Group stats -> rstd -> hn (tiny DVE/ACT chain + one small matmul)."""
        x_sb, stats = st[img]["x"], st[img]["stats"]
        # stats cast to fp16 for the projector matmul (1 cyc/row vs fp32's
        # dual-pass quarter rate); group averaging divides the fp16 rounding
        # by sqrt(32), so the rstd error stays ~1e-5.
        stats16 = stat.tile([P, 2 * CO], F16, tag="stats16")
        nc.vector.tensor_scalar(
            out=stats16[:], in0=stats[:], scalar1=1.0 / N, scalar2=None, op0=OP.mult
        )
        gs_ps = aux.tile([P, FD], F32, tag="aux")
        nc.tensor.matmul(
            gs_ps[:, 0 : 2 * CO], lhsT=proj16_sb[:], rhs=stats16[:], start=True, stop=True
        )
        m2 = stat.tile([P, CO], F32, tag="m2")
        nc.scalar.activation(m2[:], gs_ps[:, 0:CO], AF.Square)
        gs_sb = gs_ps
        ve = stat.tile([P, CO], F32, tag="ve")
        nc.vector.scalar_tensor_tensor(
            out=ve[:],
            in0=gs_sb[:, CO : 2 * CO],
            scalar=EPS,
            in1=m2[:],
            op0=OP.add,
            op1=OP.subtract,
        )
        y0i = stat.tile([P, CO], mybir.dt.int32, tag="y0i")
        nc.vector.tensor_scalar(
            out=y0i[:],
            in0=ve[:].bitcast(mybir.dt.int32),
            scalar1=1,
            scalar2=None,
            op0=OP.arith_shift_right,
        )
        nc.vector.tensor_scalar(
            out=y0i[:],
            in0=y0i[:],
            scalar1=-1,
            scalar2=0x5F3759DF,
            op0=OP.mult,
            op1=OP.add,
        )
        rstd = y0i[:].bitcast(F32)
        yy = stat.tile([P, CO], F32, tag="yy")
        nc.vector.tensor_mul(yy[:], rstd, rstd)
        nc.vector.tensor_mul(yy[:], yy[:], ve[:])
        nc.vector.tensor_scalar(
            out=yy[:], in0=yy[:], scalar1=-0.5, scalar2=1.5, op0=OP.mult, op1=OP.add
        )
        nxt = stat.tile([P, CO], F32, tag="rstd")
        nc.vector.tensor_mul(nxt[:], rstd, yy[:])
        rstd = nxt[:]
        a_sc = stat.tile([P, CO], F32, tag="a_sc")
        nc.vector.tensor_mul(a_sc[:], small["gamma"][:], rstd[:])
        bt = stat.tile([P, CO], F32, tag="bt")
        nc.vector.tensor_mul(bt[:], gs_sb[:, 0:CO], a_sc[:])
        b_sc = stat.tile([P, CO], F32, tag="b_sc")
        nc.vector.tensor_sub(b_sc[:], small["beta"][:], bt[:])

        # normalize: steady-state images go on the otherwise-idle GpSimd engine
        # (SBUF->SBUF affine, off DVE's critical eviction stream); the first two
        # images alternate DVE/ACT for latency since the pipeline ramps on them.
        hn = hns.tile([P, CO, N], F8, tag="hn")
        for co in range(CO):
            if img >= 2:
                nc.gpsimd.tensor_scalar(
                    out=hn[:, co],
                    in0=x_sb[:, co],
                    scalar1=a_sc[:, co : co + 1],
                    scalar2=b_sc[:, co : co + 1],
                    op0=OP.mult,
                    op1=OP.add,
                )
            elif co % 2 == 0:
                nc.vector.tensor_scalar(
                    out=hn[:, co],
                    in0=x_sb[:, co],
                    scalar1=a_sc[:, co : co + 1],
                    scalar2=b_sc[:, co : co + 1],
                    op0=OP.mult,
                    op1=OP.add,
                )
            else:
                nc.scalar.activation(
                    hn[:, co],
                    x_sb[:, co],
                    AF.Identity,
                    bias=b_sc[:, co : co + 1],
                    scale=a_sc[:, co : co + 1],
                )
        st[img]["hn"] = hn

    def qkv_blocks(img):
        """12 PE blocks (4 MMs each) producing q, k, vT for `img`; returned as
        closures so they can interleave with the previous image's scores."""
        hn = st[img]["hn"]
        q_sb = qs.tile([P, CO, N], F8, tag="q")
        k_sb = ks.tile([P, CO, N], F8, tag="k")
        vT = vs.tile([P, MO, C], F8, tag="vT")
        st[img]["q"] = q_sb
        st[img]["k"] = k_sb
        st[img]["vT"] = vT
        blocks = []

        def proj_block(wname, dst, ot, bname, on_act):
            def emit():
                wt = w_sb[wname]
                ps = mmp.tile([P, 2 * FD], F32, tag="mm")
                # j outer / ch inner: each DoubleRow weight pair serves two
                # moving chunks back-to-back.
                for j in range(CO // 2):
                    for ch in range(NCH):
                        nc.tensor.matmul(
                            ps[:, ch * FD : (ch + 1) * FD],
                            lhsT=wt[:, 2 * j : 2 * j + 2, ot * P : (ot + 1) * P],
                            rhs=hn[:, 2 * j : 2 * j + 2, ch * FD : (ch + 1) * FD],
                            start=(j == 0),
                            stop=(j == CO // 2 - 1),
                            perf_mode=DR,
                        )
                if on_act:
                    nc.scalar.activation(
                        dst[:, ot], ps[:], AF.Identity, bias=small[bname][:, ot : ot + 1]
                    )
                else:
                    nc.vector.tensor_scalar(
                        out=dst[:, ot],
                        in0=ps[:],
                        scalar1=small[bname][:, ot : ot + 1],
                        scalar2=None,
                        op0=OP.add,
                    )
            return emit

        def v_block(mp):
            def emit():
                ps = mmp.tile([P, 2 * FD], F32, tag="mm")
                for h in range(2):
                    mt = 2 * mp + h
                    for j in range(CO // 2):
                        nc.tensor.matmul(
                            ps[:, h * FD : (h + 1) * FD],
                            lhsT=hn[:, 2 * j : 2 * j + 2, mt * P : (mt + 1) * P],
                            rhs=w_sb["wvT"][:, 2 * j : 2 * j + 2, :],
                            start=(j == 0),
                            stop=(j == CO // 2 - 1),
                            perf_mode=DR,
                        )
                nc.vector.tensor_add(vT[:, 2 * mp : 2 * mp + 2], ps[:], bvb_sb[:])
            return emit

        for ot in range(CO):
            blocks.append(proj_block("wqT", q_sb, ot, "bq", on_act=True))
        for ot in range(CO):
            blocks.append(proj_block("wkT", k_sb, ot, "bk", on_act=False))
        for mp in range(MO // 2):
            blocks.append(v_block(mp))
        return blocks

    def scores_exp(img, filler):
        """Score matmuls + exp evictions for `img`, with `filler` blocks (the
        next image's q/k/v projections) interleaved so the PE keeps streaming
        while ACT works through the exps."""
        q_sb, k_sb = st[img]["q"], st[img]["k"]
        eT = es.tile([P, MO, N], F8, tag="eT")
        fi = 0
        for mt in range(MO):
            ps = mmp.tile([P, 2 * FD], F32, tag="mm")
            for j in range(CO // 2):
                for ch in range(NCH):
                    nc.tensor.matmul(
                        ps[:, ch * FD : (ch + 1) * FD],
                        lhsT=k_sb[:, 2 * j : 2 * j + 2, mt * P : (mt + 1) * P],
                        rhs=q_sb[:, 2 * j : 2 * j + 2, ch * FD : (ch + 1) * FD],
                        start=(j == 0),
                        stop=(j == CO // 2 - 1),
                        perf_mode=DR,
                    )
            nc.scalar.activation(eT[:, mt], ps[:], AF.Exp, scale=SC2, bias=esh_sb[:])
            while fi * MO < len(filler) * (mt + 1):
                filler[fi]()
                fi += 1
        assert fi == len(filler)
        st[img]["eT"] = eT

    def tail_out(img):
        """rowsum + out GEMM; the reciprocal and the rinv-normalizing
        evictions overlap the later out blocks."""
        vT, eT = st[img]["vT"], st[img]["eT"]
        rinv = ris.tile([P, N], F32, tag="rinv")
        out8 = ous.tile([P, CO, N], F8, tag="out")
        # rowsum first (it only needs the exps, which the out matmuls need
        # anyway): per-partition-broadcast column sums of eT via DoubleRow
        # ones-matmuls.  rinv is then ready before the first out block ends,
        # so the eviction chain starts as early as possible.
        for ch in range(NCH):
            rs = aux.tile([P, FD], F32, tag="aux")
            for jm in range(MO // 2):
                nc.tensor.matmul(
                    rs[:],
                    lhsT=ones8_sb[:],
                    rhs=eT[:, 2 * jm : 2 * jm + 2, ch * FD : (ch + 1) * FD],
                    start=(jm == 0),
                    stop=(jm == MO // 2 - 1),
                    perf_mode=DR,
                )
            nc.vector.reciprocal_approx_fast(rinv[:, ch * FD : (ch + 1) * FD], rs[:])
        pso = {}
        for ct in range(CO):
            ps = mmp.tile([P, 2 * FD], F32, tag="mm")
            pso[ct] = ps
            for jm in range(MO // 2):
                for ch in range(NCH):
                    nc.tensor.matmul(
                        ps[:, ch * FD : (ch + 1) * FD],
                        lhsT=vT[:, 2 * jm : 2 * jm + 2, ct * P : (ct + 1) * P],
                        rhs=eT[:, 2 * jm : 2 * jm + 2, ch * FD : (ch + 1) * FD],
                        start=(jm == 0),
                        stop=(jm == MO // 2 - 1),
                        perf_mode=DR,
                    )
            if ct >= 1:
                # evict ct-1 while ct's matmuls stream (rinv is ready by now)
                nc.vector.tensor_mul(out8[:, ct - 1], pso[ct - 1][:], rinv[:])
        nc.vector.tensor_mul(out8[:, CO - 1], pso[CO - 1][:], rinv[:])
        st[img]["out8"] = out8

    def tail_proj(img):
        """proj GEMM + residual + store."""
        x_sb, out8 = st[img]["x"], st[img]["out8"]
        y_ap = aps["y"][img].rearrange("(co ci) n -> ci co n", ci=P)
        for ot in range(CO):
            # The very last block of the whole kernel is evicted in two
            # 512-wide halves so the exposed eviction+DMA chain after the
            # final matmul is half as long.
            last = img == IPC - 1 and ot == CO - 1
            ps = mmp.tile([P, 2 * FD], F32, tag="mm")
            if last:
                loops = [(ch, j) for ch in range(NCH) for j in range(CO // 2)]
            else:
                loops = [(ch, j) for j in range(CO // 2) for ch in range(NCH)]
            for ch, j in loops:
                nc.tensor.matmul(
                    ps[:, ch * FD : (ch + 1) * FD],
                    lhsT=w_sb["wpT"][:, 2 * j : 2 * j + 2, ot * P : (ot + 1) * P],
                    rhs=out8[:, 2 * j : 2 * j + 2, ch * FD : (ch + 1) * FD],
                    start=(j == 0),
                    stop=(j == CO // 2 - 1),
                    perf_mode=DR,
                )
            if last:
                for ch in range(NCH):
                    t2 = ys.tile([P, FD], F32, tag="yo2")
                    nc.vector.scalar_tensor_tensor(
                        out=t2[:],
                        in0=ps[:, ch * FD : (ch + 1) * FD],
                        scalar=FS,
                        in1=x_sb[:, ot, ch * FD : (ch + 1) * FD],
                        op0=OP.mult,
                        op1=OP.add,
                    )
                    nc.sync.dma_start(y_ap[:, ot, ch * FD : (ch + 1) * FD], t2[:])
            else:
                t2 = ys.tile([P, 2 * FD], F32, tag="yo")
                nc.vector.scalar_tensor_tensor(
                    out=t2[:],
                    in0=ps[:],
                    scalar=FS,
                    in1=x_sb[:, ot],
                    op0=OP.mult,
                    op1=OP.add,
                )
                nc.sync.dma_start(y_ap[:, ot], t2[:])

    warmup(130)
    prep(0)
    prep_finish(0)   # the stats matmul lands between the two warmup batches
    warmup(90)
    wsb = stat.tile([P, P], F32, tag="warm_sb")
    nc.scalar.activation(wsb[:], wps[:, 0:P], AF.Copy)  # releases the warm slot
    nc.gpsimd.dma_start(aps["wsink"], wsb[:])
    load_weights()
    prep(1)
    for blk in qkv_blocks(0):
        blk()
    prep_finish(1)
    for img in range(IPC):
        filler = qkv_blocks(img + 1) if img + 1 < IPC else []
        scores_exp(img, filler)
        if img + 2 < IPC:
            prep(img + 2)
            prep_finish(img + 2)
        tail_out(img)
        tail_proj(img)


def _build_program():
    nc = bacc.Bacc("TRN2", target_bir_lowering=False, debug=False)
    aps = {}
    aps["x"] = nc.dram_tensor("x", [IPC, C, N], F32, kind="ExternalInput").ap()
    for name in ("wqT", "wkT", "wvT", "wpT"):
        aps[name] = nc.dram_tensor(name, [C, C], F8, kind="ExternalInput").ap()
    aps["cpack"] = nc.dram_tensor(
        "cpack", [P, 4 * CO + P + 2 * C], F32, kind="ExternalInput"
    ).ap()
    aps["y"] = nc.dram_tensor("y", [IPC, C, N], F32, kind="ExternalOutput").ap()
    aps["wsink"] = nc.dram_tensor("wsink", [P, P], F32, kind="ExternalOutput").ap()

    with tile.TileContext(nc) as tc:
        with ExitStack() as ctx:
            _emit(tc, ctx, aps)
    nc.compile()
    return nc


_PROGRAM = None


def _get_program():
    global _PROGRAM
    if _PROGRAM is None:
        _PROGRAM = _build_program()
    return _PROGRAM


def _col_layout(v):
    # (C,) vector -> [128, CO] tile layout with c = co*128 + ci at [ci, co]
    return np.ascontiguousarray(v.reshape(CO, P).T.astype(np.float32))


def _q8(w):
    return np.clip(w, -240.0, 240.0).astype(ml_dtypes.float8_e4m3)


def _make_in_maps(inputs):
    x = np.asarray(inputs["x"], dtype=np.float32).reshape(B, C, N)
    cpack = np.concatenate(
        [
            _col_layout(WS * np.asarray(inputs["bq"])),
            _col_layout(WS * np.asarray(inputs["bk"])),
            _col_layout(np.asarray(inputs["gn_gamma"])),
            _col_layout(np.asarray(inputs["gn_beta"])),
            _make_proj(),
            np.tile(
                WS * np.asarray(inputs["bv"], dtype=np.float32)[None, :], (P, 2)
            ),
        ],
        axis=1,
    )
    shared = {
        "wqT": np.ascontiguousarray(_q8(WS * np.asarray(inputs["wq"]).T)),
        "wkT": np.ascontiguousarray(_q8(WS * np.asarray(inputs["wk"]).T)),
        "wvT": np.ascontiguousarray(_q8(WS * np.asarray(inputs["wv"]).T)),
        "wpT": np.ascontiguousarray(_q8(WS * np.asarray(inputs["wp"]).T)),
        "cpack": np.ascontiguousarray(cpack),
    }
    in_maps = []
    for core in range(NCORES):
        m = dict(shared)
        m["x"] = np.ascontiguousarray(x[core * IPC : (core + 1) * IPC])
        in_maps.append(m)
    return in_maps


def _make_proj():
    # [128,128] group-averaging projector: P[i,j] = (i//32 == j//32) / 32
    # (channel c = co*128 + ci; each co slab holds 4 groups of 32 channels).
    # The kernel pre-scales the (sum, sumsq) stats by 1/N before this matmul,
    # and the fp16 copy of this matrix needs 1/32 to stay in normal range.
    gsz = P // (GROUPS // CO)  # 32
    idx = np.arange(P) // gsz
    return np.ascontiguousarray((idx[:, None] == idx[None, :]).astype(np.float32) / gsz)


def _run(inputs, trace=False):
    nc = _get_program()
    in_maps = _make_in_maps(inputs)
    res = run_bass_kernel_spmd(nc, in_maps, core_ids=list(range(NCORES)), trace=trace)
    y = np.concatenate([r["y"] for r in res.results], axis=0)  # (B, C, N)
    bp = np.asarray(inputs["bp"], dtype=np.float32)
    if np.any(bp):
        y = y + bp[None, :, None]
    return y.reshape(B, C, H, W).astype(np.float32), res.exec_time_ns


def kernel(**inputs):
    return _run(inputs, trace=False)[0]
